# revision 43
# baseline (speedup 1.0000x reference)
"""Multi-head attention (B=8, N=1024, C=768, 12 heads) on 8 TRN2 NeuronCores.

Sharding: data-parallel over batch — batch element b runs on core b, weights
replicated, zero collectives.

Per-core kernel (all matmuls bf16 on the TensorEngine):
  - Host pre-transposes AND pre-converts x, W_qkv, W_proj to bf16, so tiles
    DMA straight into matmul-ready SBUF layout: no on-device casts, no
    staging bounce, and half the input DMA bytes. (Numerically identical to
    casting on device — every matmul consumed bf16 anyway.)
  - DMAs are issued in consumption order so the first qkv chain starts
    ~1us in; the PE ramps its clock on real work instead of a scratch
    warm-up.
  - qkv: q^T,k^T [768,1024] and v [1024,768] via 6-chunk K=768 matmuls.
  - scores are computed TRANSPOSED per head: S^T[k,q] with lhsT=k^T-block,
    rhs=q^T-block, so the exp output P^T feeds the P@V matmul directly as
    the moving operand (no transpose of the attention matrix). The
    1/sqrt(d) scale rides for free on the exp's affine pre-scale.
  - softmax denominators come free: v is stored with a ones-column
    appended per head (lhsT [128,65]); row 64 of the P@V accumulator is
    sum_k exp(S), i.e. the denominator.
  - normalization runs entirely off the TensorEngine's critical path:
    copy the accumulator to SBUF (releasing its PSUM slot), fast
    approximate reciprocal on VectorE, broadcast across partitions on the
    (otherwise idle) GpSimd engine, one elementwise multiply per head.
  - qkv chunk emission is interleaved into the attention stream as
    filler so the TensorEngine stays dense while ScalarE works through
    the exps.
  - proj: y = attn @ W_proj^T + b_proj, bias materialized once via
    partition_broadcast and added during the PSUM->SBUF staging. The
    c<5 accumulations of all 8 n-chunks run first (absorbing head 11's
    normalize latency); the closers then drain per-chunk with the
    bias-add + store pipelined behind them so the kernel tail is one
    chunk deep, not a whole group.
"""

from contextlib import ExitStack

import numpy as np

import concourse.mybir as mybir
import concourse.tile as tile
from concourse import bacc
from concourse.bass_utils import run_bass_kernel_spmd

B, N, C = 8, 1024, 768
NH, D = 12, 64
CK = C // 128  # 6 contraction chunks of 128
NQ = N // 128  # 8 position chunks of 128
SCALE = D ** -0.5
F32 = mybir.dt.float32
BF16 = mybir.dt.bfloat16
Exp = mybir.ActivationFunctionType.Exp


def _emit(tc, xT, wqkvT, wprojT, bproj, out):
    nc = tc.nc
    with ExitStack() as ctx:
        sb = ctx.enter_context(tc.tile_pool(name="sb", bufs=1))
        pp = ctx.enter_context(tc.tile_pool(name="pp", bufs=8))
        small = ctx.enter_context(tc.tile_pool(name="small", bufs=2))
        yst = ctx.enter_context(tc.tile_pool(name="yst", bufs=3))
        # PSUM pools are released by hand: qkv+attention use ps/acc, the
        # projection reuses the freed banks for a deeper y pipeline.
        ps = tc.alloc_tile_pool(name="ps", bufs=3, space="PSUM")
        acc = tc.alloc_tile_pool(name="acc", bufs=1, space="PSUM")

        # ---- input tiles (DMA straight into matmul layout) ------------
        # Weights are host-merged into partition-major [128, (c, cols)]
        # layouts so whole priority classes load with ONE strided DMA
        # (fewer DMAs -> fewer semaphores -> shorter end-of-graph drain
        # and less DGE issue traffic). Host also pre-permutes wqkv
        # columns into priority groups: [0:768) = q-left|k-left,
        # [768:1536) = v, [1536:2304) = q-right|k-right.
        xT_bf = [
            sb.tile([128, N], BF16, name=f"xT_bf{c}", tag=f"xT_bf{c}")
            for c in range(CK)
        ]
        wq_all = sb.tile([128, CK, 3 * C], BF16, name="wq_all", tag="wq_all")
        wp_all = sb.tile([128, CK, C], BF16, name="wp_all", tag="wp_all")
        wqkvT_v = wqkvT.rearrange("p (c k) -> p c k", k=3 * C)
        wprojT_v = wprojT.rearrange("p (c k) -> p c k", k=C)

        # Issue order == consumption order: q-left/k-left weights and x
        # feed the opening qkv chains (per-c DMAs so the first chain
        # starts after ~one chunk); v weights must land before the v
        # fillers in head 0; right halves ride through the first few
        # attention heads; W_proj and the bias are only needed at the
        # projection, ~150us in. x issues from the (otherwise idle)
        # Scalar DGE so x and W descriptor generation run in parallel.
        for c in range(CK):
            nc.scalar.dma_start(out=xT_bf[c][:], in_=xT[c * 128:(c + 1) * 128, :])
            nc.sync.dma_start(out=wq_all[:, c, 0:256], in_=wqkvT_v[:, c, 0:256])
        # v weights next (head 0's v fillers start ~22us in), in halves
        # so the first emit_v chain isn't gated on the full block
        nc.sync.dma_start(out=wq_all[:, :, 768:1152], in_=wqkvT_v[:, :, 768:1152])
        nc.sync.dma_start(out=wq_all[:, :, 1152:1536], in_=wqkvT_v[:, :, 1152:1536])
        for c in range(CK):
            nc.sync.dma_start(out=wq_all[:, c, 256:768], in_=wqkvT_v[:, c, 256:768])
        nc.sync.dma_start(out=wq_all[:, :, 1536:2304], in_=wqkvT_v[:, :, 1536:2304])
        nc.sync.dma_start(out=wp_all[:], in_=wprojT_v[:, :, :])
        # bias pre-replicated host-side to all 128 partitions
        bias_bc = sb.tile([128, C], F32, name="bias_bc", tag="bias_bc")
        nc.sync.dma_start(out=bias_bc[:], in_=bproj[:, :])

        # ---- qkv projections -----------------------------------------
        # q^T,k^T: chunk m covers rows [m*128,(m+1)*128) of qkv^T;
        # m in 0..5 -> q, m in 6..11 -> k.
        qkT = [
            sb.tile([128, N], BF16, name=f"qkT{m}", tag=f"qkT{m}")
            for m in range(12)
        ]

        # column offset of qkv^T row-block m in the host-permuted W
        # layout: [m0 | m6 | m1 m2 | m7 m8 | v | m3 m4 m5 | m9 m10 m11]
        # (m0+m6 lead so the first DMA wave carries exactly the two
        # blocks head 0 needs)
        def wcol(m):
            if m == 0:
                return 0
            if m == 6:
                return 128
            if m in (1, 2):
                return 256 + (m - 1) * 128
            if m in (7, 8):
                return 512 + (m - 7) * 128
            if m < 6:
                return 1536 + (m - 3) * 128  # q-right
            return 1920 + (m - 9) * 128  # k-right

        def emit_qk_h(m, qh, copy_on_scalar=False, warm_fill=0):
            qk_ps = ps.tile([128, 512], F32, name=f"qk_ps{m}_{qh}", tag="s")
            for c in range(CK):
                nc.tensor.matmul(
                    qk_ps[:],
                    lhsT=wq_all[:, c, wcol(m):wcol(m) + 128],
                    rhs=xT_bf[c][:, qh * 512:(qh + 1) * 512],
                    start=(c == 0),
                    stop=(c == CK - 1),
                )
                if c < warm_fill:
                    # scratch matmul between DMA-paced chain links: keeps
                    # the PE busy (clock ramping) while the next x chunk
                    # streams in
                    nc.tensor.matmul(
                        warm_ps[:],
                        lhsT=warm_in[:, 0:128],
                        rhs=warm_in[:],
                        start=True,
                        stop=True,
                    )
            if copy_on_scalar:
                # ScalarE is idle until the first exp; its Copy is also
                # faster than DVE's here, halving the PSUM->SBUF chain
                # that gates head 0's first scores.
                nc.scalar.copy(qkT[m][:, qh * 512:(qh + 1) * 512], qk_ps[:])
            else:
                nc.vector.tensor_copy(qkT[m][:, qh * 512:(qh + 1) * 512], qk_ps[:])

        def emit_qk(m):
            for qh in range(2):
                emit_qk_h(m, qh)

        # v in natural layout [n, (head, d)] with a ones column appended
        # per head: v_sb[n] is [128, NH, D+1], [:, h, D] == 1.0.
        v_sb = [
            sb.tile([128, NH, D + 1], BF16, name=f"v_sb{n}", tag=f"v_sb{n}")
            for n in range(NQ)
        ]

        def emit_v(n):
            nc.vector.memset(v_sb[n][:, :, D], 1.0)
            for half in range(2):
                v_ps = ps.tile([128, 384], F32, name=f"v_ps{n}_{half}", tag="s")
                for c in range(CK):
                    nc.tensor.matmul(
                        v_ps[:],
                        lhsT=xT_bf[c][:, n * 128:(n + 1) * 128],
                        rhs=wq_all[:, c, 768 + half * 384:768 + (half + 1) * 384],
                        start=(c == 0),
                        stop=(c == CK - 1),
                    )
                nc.vector.tensor_copy(
                    v_sb[n][:, half * 6:(half + 1) * 6, 0:D],
                    v_ps[:].rearrange("p (h d) -> p h d", d=D),
                )

        # ---- attention ------------------------------------------------
        attn_bf = [
            sb.tile([128, N], BF16, name=f"attn_bf{p}", tag=f"attn_bf{p}")
            for p in range(6)
        ]

        def emit_head(h, filler=None):
            """S^T + exp + P@V for head h; `filler` emits extra PE work
            early in the stream (previous head's deferred normalize, next
            qkv chunk) so PE has exp-independent work while ScalarE runs."""
            q_tile = qkT[h // 2]
            k_tile = qkT[6 + h // 2]
            ro = (h % 2) * 64
            out_aug = acc.tile([D + 1, N], F32, name=f"oaug{h}", tag="acc")

            def emit_S(kc):
                st = ps.tile([128, N], F32, name=f"s{h}_{kc}", tag="s")
                for qh in range(2):
                    nc.tensor.matmul(
                        st[:, qh * 512:(qh + 1) * 512],
                        lhsT=k_tile[ro:ro + D, kc * 128:(kc + 1) * 128],
                        rhs=q_tile[ro:ro + D, qh * 512:(qh + 1) * 512],
                        start=True,
                        stop=True,
                    )
                pt = pp.tile([128, N], BF16, name=f"P{h}_{kc}", tag="P")
                nc.scalar.activation(pt[:], st[:], Exp, scale=SCALE)
                return pt

            def emit_v_mm(kc, pt):
                for qh in range(2):
                    nc.tensor.matmul(
                        out_aug[:, qh * 512:(qh + 1) * 512],
                        lhsT=v_sb[kc][:, h, :],
                        rhs=pt[:, qh * 512:(qh + 1) * 512],
                        start=(kc == 0),
                        stop=(kc == NQ - 1),
                    )

            # software pipeline: exp(kc) overlaps S(kc+1) and P@V(kc-1)
            pts = {0: emit_S(0), 1: emit_S(1)}
            if filler is not None:
                filler()
            for kc in range(NQ):
                emit_v_mm(kc, pts.pop(kc))
                if kc + 2 < NQ:
                    pts[kc + 2] = emit_S(kc + 2)
            return out_aug

        def emit_norm_pre(h, oa, direct=False):
            """DVE/GpSimd-only part: reciprocal chain first (it gates the
            final multiply), then stage the accumulator to SBUF to release
            its PSUM slot. For the last head (`direct`) the multiply reads
            the accumulator straight from PSUM instead — shortest tail."""
            if not direct:
                # staging copy FIRST: it releases the single-slot PSUM
                # accumulator, which gates the next head's P@V matmuls.
                # Split in halves so the first starts right after the
                # P@V qh0 matmul, before qh1 has finished.
                un = small.tile([D, N], F32, name=f"un{h}", tag="un")
                nc.vector.tensor_copy(un[:, 0:512], oa[0:D, 0:512])
                nc.vector.tensor_copy(un[:, 512:N], oa[0:D, 512:N])
            dn = small.tile([1, N], F32, name=f"dn{h}", tag="dn")
            nc.vector.tensor_copy(dn[:], oa[D:D + 1, :])
            rc = small.tile([1, N], F32, name=f"rc{h}", tag="rc")
            # reciprocal_approx_fast's uOp program only works from
            # partition 0 on hardware, hence the dn bounce copy above.
            nc.vector.reciprocal_approx_fast(rc[:], dn[:])
            rcb = small.tile([1, N], BF16, name=f"rcb{h}", tag="rcb")
            nc.vector.tensor_copy(rcb[:], rc[:])
            bcast = small.tile([64, N], BF16, name=f"bcast{h}", tag="bcast")
            if direct:
                # last head: the chain below gates the proj closers, so
                # pipeline broadcast and multiply in halves
                nc.gpsimd.partition_broadcast(bcast[:, 0:512], rcb[:, 0:512])
                nc.gpsimd.partition_broadcast(bcast[:, 512:N], rcb[:, 512:N])
                return oa, bcast
            nc.gpsimd.partition_broadcast(bcast[:], rcb[:])
            return un, bcast

        def emit_norm_post(h, un, bcast):
            p, ro = h // 2, (h % 2) * 64
            if h == NH - 1:
                for lo, hi in ((0, 512), (512, N)):
                    nc.vector.tensor_mul(
                        attn_bf[p][ro:ro + 64, lo:hi], un[0:D, lo:hi], bcast[:, lo:hi]
                    )
            else:
                nc.vector.tensor_mul(attn_bf[p][ro:ro + 64, :], un[0:D, :], bcast[:])

        # ---- PE p-state warm-up --------------------------------------
        # The PE clock ramps with continuous busy time; the first ~5us
        # of the kernel are DMA lead-in with nothing else for the PE to
        # do, so a short scratch-matmul burst there ramps the clock
        # before the first real chains arrive (cold matmuls run ~2.7x
        # slow). Seeded by a DVE memset — no DMA dependency.
        warm_in = sb.tile([128, 512], BF16, name="warm_in", tag="warm_in")
        nc.vector.memset(warm_in[:], 1.0)
        warm_ps = ps.tile([128, 512], F32, name="warm_ps", tag="s")
        for i in range(6):
            nc.tensor.matmul(
                warm_ps[:],
                lhsT=warm_in[:, 0:128],
                rhs=warm_in[:],
                start=(i == 0),
                stop=(i == 5),
            )

        # v(0) rides as head 0's first filler — S(0)/S(1) only need
        # qkT[0]/qkT[6], so head 0 starts without waiting on v weights.
        # The k-side copies run on ScalarE so the m0/m6 staging copies
        # proceed two at a time.
        emit_qk_h(0, 0)
        emit_qk_h(6, 0, copy_on_scalar=True)
        emit_qk_h(0, 1)
        emit_qk_h(6, 1, copy_on_scalar=True)

        # Remaining qkv work rides inside the attention stream as PE
        # filler during exp waits: head 0 carries the other v chunks
        # (needed from its own P@V loop onward); later heads each carry
        # one q/k chunk, landing one pair ahead of first use.
        QK_FILL = {1: (1, 7), 2: (2,), 3: (8,), 4: (3,), 5: (9,),
                   6: (4,), 7: (10,), 8: (5,), 9: (11,)}

        pending = None
        for h in range(NH):
            fillers = []
            if h == 0:
                fillers.append(lambda: [emit_v(n) for n in range(NQ)])
            for m in QK_FILL.get(h, ()):
                fillers.append(lambda m=m: emit_qk(m))
            if pending is not None:
                ph, un, bc = pending
                fillers.append(lambda ph=ph, un=un, bc=bc: emit_norm_post(ph, un, bc))

            def filler():
                for f in fillers:
                    f()

            oa = emit_head(h, filler=filler)
            pending = (h, *emit_norm_pre(h, oa, direct=(h == NH - 1)))
        emit_norm_post(*pending)

        # ---- output projection ---------------------------------------
        acc.release()
        ps.release()
        yps = tc.alloc_tile_pool(name="yps", bufs=3, space="PSUM")

        # The first three n-chunks accumulate c=0..4 up front — ~5us of
        # matmuls independent of attn_bf[5] that absorb head 11's
        # normalize-chain latency — and get their c=5 closer deferred.
        # By n=3 the chain has resolved, so the rest run as plain
        # 6-chunk accumulations with the bias-add + store pipelined
        # per-chunk behind them: the kernel tail is one chunk deep.
        y_tiles = {}
        for n in range(3):
            y_ps = yps.tile([128, C], F32, name=f"y_ps{n}", tag="y_ps")
            y_tiles[n] = y_ps
            for lo, hi in ((0, 512), (512, 768)):
                for c in range(CK - 1):
                    nc.tensor.matmul(
                        y_ps[:, lo:hi],
                        lhsT=attn_bf[c][:, n * 128:(n + 1) * 128],
                        rhs=wp_all[:, c, lo:hi],
                        start=(c == 0),
                        stop=False,
                    )
        for n in range(NQ):
            if n < 3:
                y_ps = y_tiles[n]
                for lo, hi in ((0, 512), (512, 768)):
                    nc.tensor.matmul(
                        y_ps[:, lo:hi],
                        lhsT=attn_bf[CK - 1][:, n * 128:(n + 1) * 128],
                        rhs=wp_all[:, CK - 1, lo:hi],
                        start=False,
                        stop=True,
                    )
            else:
                y_ps = yps.tile([128, C], F32, name=f"y_ps{n}", tag="y_ps")
                for lo, hi in ((0, 512), (512, 768)):
                    for c in range(CK):
                        nc.tensor.matmul(
                            y_ps[:, lo:hi],
                            lhsT=attn_bf[c][:, n * 128:(n + 1) * 128],
                            rhs=wp_all[:, c, lo:hi],
                            start=(c == 0),
                            stop=(c == CK - 1),
                        )
            y_sb = yst.tile([128, C], F32, name=f"y_sb{n}", tag="y")
            nc.vector.tensor_add(y_sb[:], y_ps[:], bias_bc[:])
            nc.sync.dma_start(out=out[n * 128:(n + 1) * 128, :], in_=y_sb[:])
        yps.release()


def build_graph():
    nc = bacc.Bacc("TRN2", target_bir_lowering=False, debug=False)
    xT = nc.declare_dram_parameter("xT", [C, N], BF16, isOutput=False)
    wqkvT = nc.declare_dram_parameter("wqkvT", [128, CK * 3 * C], BF16, isOutput=False)
    wprojT = nc.declare_dram_parameter("wprojT", [128, CK * C], BF16, isOutput=False)
    bproj = nc.declare_dram_parameter("bproj", [128, C], F32, isOutput=False)
    out = nc.declare_dram_parameter("out", [N, C], F32, isOutput=True)
    with tile.TileContext(nc) as tc:
        _emit(tc, xT.ap(), wqkvT.ap(), wprojT.ap(), bproj.ap(), out.ap())
    nc.compile()
    return nc


_GRAPH = None


def _get_graph():
    global _GRAPH
    if _GRAPH is None:
        _GRAPH = build_graph()
    return _GRAPH


def make_in_maps(x, W_qkv, W_proj, b_proj):
    import ml_dtypes

    bf16 = ml_dtypes.bfloat16
    x = np.asarray(x, dtype=np.float32)
    wqkvT = np.asarray(W_qkv, dtype=np.float32).T.astype(bf16)  # [768, 2304]
    # permute columns into priority-contiguous groups:
    # [m0 | m6 | m1 m2 | m7 m8 | v | m3 m4 m5 | m9 m10 m11] (see wcol)
    perm = np.concatenate([
        np.arange(0, 128),          # q m0
        np.arange(768, 896),        # k m6
        np.arange(128, 384),        # q m1,m2
        np.arange(896, 1152),       # k m7,m8
        np.arange(1536, 2304),      # v
        np.arange(384, 768),        # q m3..5
        np.arange(1152, 1536),      # k m9..11
    ])
    wqkvT = wqkvT[:, perm]
    # merge to partition-major [128, c, cols] so each priority class is
    # one strided DMA on device
    wqkvT = np.ascontiguousarray(
        wqkvT.reshape(CK, 128, 3 * C).transpose(1, 0, 2).reshape(128, CK * 3 * C)
    )
    wprojT = np.asarray(W_proj, dtype=np.float32).T.astype(bf16)
    wprojT = np.ascontiguousarray(
        wprojT.reshape(CK, 128, C).transpose(1, 0, 2).reshape(128, CK * C)
    )
    bp = np.ascontiguousarray(
        np.broadcast_to(np.asarray(b_proj, dtype=np.float32), (128, C))
    )
    xT_all = np.ascontiguousarray(x.transpose(0, 2, 1).astype(bf16))
    return [
        {"xT": xT_all[i], "wqkvT": wqkvT, "wprojT": wprojT, "bproj": bp}
        for i in range(B)
    ]


def run(x, W_qkv, W_proj, b_proj, trace=False):
    nc = _get_graph()
    in_maps = make_in_maps(x, W_qkv, W_proj, b_proj)
    res = run_bass_kernel_spmd(nc, in_maps, core_ids=list(range(B)), trace=trace)
    out = np.stack([res.results[i]["out"] for i in range(B)], axis=0)
    return out.astype(np.float32, copy=False), res


def kernel(x, W_qkv, W_proj, b_proj, H=None, W=None):
    out, _ = run(x, W_qkv, W_proj, b_proj)
    return out


# revision 44
# speedup vs baseline: 1.1863x; 1.1863x over previous
"""Multi-head attention (B=8, N=1024, C=768, 12 heads) on 8 TRN2 NeuronCores.

Sharding: data-parallel over batch — batch element b runs on core b, weights
replicated, zero collectives.

Per-core kernel (all matmuls bf16 on the TensorEngine):
  - Host pre-transposes AND pre-converts x, W_qkv, W_proj to bf16, permutes
    W_qkv columns into DMA-priority order, merges the weight chunks into
    partition-major blocks, and replicates the bias to 128 partitions.
    Tiles then DMA straight into matmul-ready SBUF layout: no on-device
    casts, no staging bounce, half the input DMA bytes, and few DMA
    instructions (each dma_start costs ~600ns of DGE issue and one
    semaphore that the end-of-graph drain walks). Numerically identical to
    casting on device — every matmul consumed bf16 anyway.
  - DMAs are issued in consumption order ([x(c)+W m0|m6] -> v -> other
    left blocks -> right blocks -> W_proj -> bias), x from the Scalar
    engine's DGE and W from Sync so descriptor generation runs in
    parallel. A 6-matmul scratch burst ramps the PE clock through the
    DMA lead-in (cold matmuls run at half clock for several us).
  - qkv: q^T,k^T [768,1024] and v [1024,768] via 6-chunk K=768 matmuls.
  - scores are computed TRANSPOSED per head: S^T[k,q] with lhsT=k^T-block,
    rhs=q^T-block, so the exp output P^T feeds the P@V matmul directly as
    the moving operand (no transpose of the attention matrix). The
    1/sqrt(d) scale rides for free on the exp's affine pre-scale.
  - softmax denominators come free: v is stored with a ones-column
    appended per head (lhsT [128,65]); row 64 of the P@V accumulator is
    sum_k exp(S), i.e. the denominator.
  - normalization runs entirely off the TensorEngine's critical path:
    copy the accumulator to SBUF in halves (releasing its PSUM slot as
    early as possible — it gates the next head's P@V), fast approximate
    reciprocal on VectorE, broadcast across partitions on the (otherwise
    idle) GpSimd engine, one elementwise multiply per head. The last
    head multiplies straight out of PSUM with broadcast and multiply
    split in halves — that chain gates the projection closers.
  - qkv chunk emission is interleaved into the attention stream as
    filler so the TensorEngine stays dense while ScalarE works through
    the exps.
  - proj: y = attn @ W_proj^T + b_proj. The first three n-chunks
    accumulate c=0..4 up front (PE work independent of head 11,
    absorbing its normalize-chain latency) with deferred c=5 closers;
    the rest run as plain 6-chunk accumulations with the bias-add +
    store pipelined per-chunk, so the kernel tail is one chunk deep.
"""

from contextlib import ExitStack

import numpy as np

import concourse.mybir as mybir
import concourse.tile as tile
from concourse import bacc
from concourse.bass_utils import run_bass_kernel_spmd

B, N, C = 8, 1024, 768
NH, D = 12, 64
CK = C // 128  # 6 contraction chunks of 128
NQ = N // 128  # 8 position chunks of 128
SCALE = D ** -0.5
F32 = mybir.dt.float32
BF16 = mybir.dt.bfloat16
Exp = mybir.ActivationFunctionType.Exp


def _emit(tc, xT, wqkvT, wprojT, bproj, out):
    nc = tc.nc
    with ExitStack() as ctx:
        sb = ctx.enter_context(tc.tile_pool(name="sb", bufs=1))
        pp = ctx.enter_context(tc.tile_pool(name="pp", bufs=8))
        small = ctx.enter_context(tc.tile_pool(name="small", bufs=2))
        yst = ctx.enter_context(tc.tile_pool(name="yst", bufs=3))
        # PSUM pools are released by hand: qkv+attention use ps/acc, the
        # projection reuses the freed banks for a deeper y pipeline.
        ps = tc.alloc_tile_pool(name="ps", bufs=3, space="PSUM")
        acc = tc.alloc_tile_pool(name="acc", bufs=1, space="PSUM")

        # ---- input tiles (DMA straight into matmul layout) ------------
        # Weights are host-merged into partition-major [128, (c, cols)]
        # layouts so whole priority classes load with ONE strided DMA
        # (fewer DMAs -> fewer semaphores -> shorter end-of-graph drain
        # and less DGE issue traffic). Host also pre-permutes wqkv
        # columns into priority groups: [0:768) = q-left|k-left,
        # [768:1536) = v, [1536:2304) = q-right|k-right.
        xT_bf = [
            sb.tile([128, N], BF16, name=f"xT_bf{c}", tag=f"xT_bf{c}")
            for c in range(CK)
        ]
        wq_all = sb.tile([128, CK, 3 * C], BF16, name="wq_all", tag="wq_all")
        wp_all = sb.tile([128, CK, C], BF16, name="wp_all", tag="wp_all")
        wqkvT_v = wqkvT.rearrange("p (c k) -> p c k", k=3 * C)
        wprojT_v = wprojT.rearrange("p (c k) -> p c k", k=C)

        # Issue order == consumption order: q-left/k-left weights and x
        # feed the opening qkv chains (per-c DMAs so the first chain
        # starts after ~one chunk); v weights must land before the v
        # fillers in head 0; right halves ride through the first few
        # attention heads; W_proj and the bias are only needed at the
        # projection, ~150us in. x issues from the (otherwise idle)
        # Scalar DGE so x and W descriptor generation run in parallel.
        for c in range(CK):
            nc.scalar.dma_start(out=xT_bf[c][:], in_=xT[c * 128:(c + 1) * 128, :])
            nc.sync.dma_start(out=wq_all[:, c, 0:256], in_=wqkvT_v[:, c, 0:256])
        # v weights next (head 0's v fillers start ~22us in), in halves
        # so the first emit_v chain isn't gated on the full block
        nc.sync.dma_start(out=wq_all[:, :, 768:1152], in_=wqkvT_v[:, :, 768:1152])
        nc.sync.dma_start(out=wq_all[:, :, 1152:1536], in_=wqkvT_v[:, :, 1152:1536])
        for c in range(CK):
            nc.sync.dma_start(out=wq_all[:, c, 256:768], in_=wqkvT_v[:, c, 256:768])
        nc.sync.dma_start(out=wq_all[:, :, 1536:2304], in_=wqkvT_v[:, :, 1536:2304])
        nc.sync.dma_start(out=wp_all[:], in_=wprojT_v[:, :, :])
        # bias pre-replicated host-side to all 128 partitions
        bias_bc = sb.tile([128, C], F32, name="bias_bc", tag="bias_bc")
        nc.sync.dma_start(out=bias_bc[:], in_=bproj[:, :])

        # ---- qkv projections -----------------------------------------
        # q^T,k^T: chunk m covers rows [m*128,(m+1)*128) of qkv^T;
        # m in 0..5 -> q, m in 6..11 -> k.
        qkT = [
            sb.tile([128, N], BF16, name=f"qkT{m}", tag=f"qkT{m}")
            for m in range(12)
        ]

        # column offset of qkv^T row-block m in the host-permuted W
        # layout: [m0 | m6 | m1 m2 | m7 m8 | v | m3 m4 m5 | m9 m10 m11]
        # (m0+m6 lead so the first DMA wave carries exactly the two
        # blocks head 0 needs)
        def wcol(m):
            if m == 0:
                return 0
            if m == 6:
                return 128
            if m in (1, 2):
                return 256 + (m - 1) * 128
            if m in (7, 8):
                return 512 + (m - 7) * 128
            if m < 6:
                return 1536 + (m - 3) * 128  # q-right
            return 1920 + (m - 9) * 128  # k-right

        def emit_qk_h(m, qh, copy_on_scalar=False, warm_fill=0):
            qk_ps = ps.tile([128, 512], F32, name=f"qk_ps{m}_{qh}", tag="s")
            for c in range(CK):
                nc.tensor.matmul(
                    qk_ps[:],
                    lhsT=wq_all[:, c, wcol(m):wcol(m) + 128],
                    rhs=xT_bf[c][:, qh * 512:(qh + 1) * 512],
                    start=(c == 0),
                    stop=(c == CK - 1),
                )
                if c < warm_fill:
                    # scratch matmul between DMA-paced chain links: keeps
                    # the PE busy (clock ramping) while the next x chunk
                    # streams in
                    nc.tensor.matmul(
                        warm_ps[:],
                        lhsT=warm_in[:, 0:128],
                        rhs=warm_in[:],
                        start=True,
                        stop=True,
                    )
            if copy_on_scalar:
                # ScalarE is idle until the first exp; its Copy is also
                # faster than DVE's here, halving the PSUM->SBUF chain
                # that gates head 0's first scores.
                nc.scalar.copy(qkT[m][:, qh * 512:(qh + 1) * 512], qk_ps[:])
            else:
                nc.vector.tensor_copy(qkT[m][:, qh * 512:(qh + 1) * 512], qk_ps[:])

        def emit_qk(m):
            for qh in range(2):
                emit_qk_h(m, qh)

        # v in natural layout [n, (head, d)] with a ones column appended
        # per head: v_sb[n] is [128, NH, D+1], [:, h, D] == 1.0.
        v_sb = [
            sb.tile([128, NH, D + 1], BF16, name=f"v_sb{n}", tag=f"v_sb{n}")
            for n in range(NQ)
        ]

        def emit_v(n):
            nc.vector.memset(v_sb[n][:, :, D], 1.0)
            for half in range(2):
                v_ps = ps.tile([128, 384], F32, name=f"v_ps{n}_{half}", tag="s")
                for c in range(CK):
                    nc.tensor.matmul(
                        v_ps[:],
                        lhsT=xT_bf[c][:, n * 128:(n + 1) * 128],
                        rhs=wq_all[:, c, 768 + half * 384:768 + (half + 1) * 384],
                        start=(c == 0),
                        stop=(c == CK - 1),
                    )
                nc.vector.tensor_copy(
                    v_sb[n][:, half * 6:(half + 1) * 6, 0:D],
                    v_ps[:].rearrange("p (h d) -> p h d", d=D),
                )

        # ---- attention ------------------------------------------------
        attn_bf = [
            sb.tile([128, N], BF16, name=f"attn_bf{p}", tag=f"attn_bf{p}")
            for p in range(6)
        ]

        def emit_head(h, filler=None):
            """S^T + exp + P@V for head h; `filler` emits extra PE work
            early in the stream (previous head's deferred normalize, next
            qkv chunk) so PE has exp-independent work while ScalarE runs."""
            q_tile = qkT[h // 2]
            k_tile = qkT[6 + h // 2]
            ro = (h % 2) * 64
            out_aug = acc.tile([D + 1, N], F32, name=f"oaug{h}", tag="acc")

            def emit_S(kc):
                st = ps.tile([128, N], F32, name=f"s{h}_{kc}", tag="s")
                for qh in range(2):
                    nc.tensor.matmul(
                        st[:, qh * 512:(qh + 1) * 512],
                        lhsT=k_tile[ro:ro + D, kc * 128:(kc + 1) * 128],
                        rhs=q_tile[ro:ro + D, qh * 512:(qh + 1) * 512],
                        start=True,
                        stop=True,
                    )
                pt = pp.tile([128, N], BF16, name=f"P{h}_{kc}", tag="P")
                nc.scalar.activation(pt[:], st[:], Exp, scale=SCALE)
                return pt

            def emit_v_mm(kc, pt):
                for qh in range(2):
                    nc.tensor.matmul(
                        out_aug[:, qh * 512:(qh + 1) * 512],
                        lhsT=v_sb[kc][:, h, :],
                        rhs=pt[:, qh * 512:(qh + 1) * 512],
                        start=(kc == 0),
                        stop=(kc == NQ - 1),
                    )

            # software pipeline: exp(kc) overlaps S(kc+1) and P@V(kc-1)
            pts = {0: emit_S(0), 1: emit_S(1)}
            if filler is not None:
                filler()
            for kc in range(NQ):
                emit_v_mm(kc, pts.pop(kc))
                if kc + 2 < NQ:
                    pts[kc + 2] = emit_S(kc + 2)
            return out_aug

        def emit_norm_pre(h, oa, direct=False):
            """DVE/GpSimd-only part: reciprocal chain first (it gates the
            final multiply), then stage the accumulator to SBUF to release
            its PSUM slot. For the last head (`direct`) the multiply reads
            the accumulator straight from PSUM instead — shortest tail."""
            if not direct:
                # staging copy FIRST: it releases the single-slot PSUM
                # accumulator, which gates the next head's P@V matmuls.
                # Split in halves so the first starts right after the
                # P@V qh0 matmul, before qh1 has finished.
                un = small.tile([D, N], F32, name=f"un{h}", tag="un")
                nc.vector.tensor_copy(un[:, 0:512], oa[0:D, 0:512])
                nc.vector.tensor_copy(un[:, 512:N], oa[0:D, 512:N])
            dn = small.tile([1, N], F32, name=f"dn{h}", tag="dn")
            nc.vector.tensor_copy(dn[:], oa[D:D + 1, :])
            rc = small.tile([1, N], F32, name=f"rc{h}", tag="rc")
            # reciprocal_approx_fast's uOp program only works from
            # partition 0 on hardware, hence the dn bounce copy above.
            nc.vector.reciprocal_approx_fast(rc[:], dn[:])
            rcb = small.tile([1, N], BF16, name=f"rcb{h}", tag="rcb")
            nc.vector.tensor_copy(rcb[:], rc[:])
            bcast = small.tile([64, N], BF16, name=f"bcast{h}", tag="bcast")
            if direct:
                # last head: the chain below gates the proj closers, so
                # pipeline broadcast and multiply in halves
                nc.gpsimd.partition_broadcast(bcast[:, 0:512], rcb[:, 0:512])
                nc.gpsimd.partition_broadcast(bcast[:, 512:N], rcb[:, 512:N])
                return oa, bcast
            nc.gpsimd.partition_broadcast(bcast[:], rcb[:])
            return un, bcast

        def emit_norm_post(h, un, bcast):
            p, ro = h // 2, (h % 2) * 64
            if h == NH - 1:
                for lo, hi in ((0, 512), (512, N)):
                    nc.vector.tensor_mul(
                        attn_bf[p][ro:ro + 64, lo:hi], un[0:D, lo:hi], bcast[:, lo:hi]
                    )
            else:
                nc.vector.tensor_mul(attn_bf[p][ro:ro + 64, :], un[0:D, :], bcast[:])

        # ---- PE p-state warm-up --------------------------------------
        # The PE clock ramps with continuous busy time; the first ~5us
        # of the kernel are DMA lead-in with nothing else for the PE to
        # do, so a short scratch-matmul burst there ramps the clock
        # before the first real chains arrive (cold matmuls run ~2.7x
        # slow). Seeded by a DVE memset — no DMA dependency.
        warm_in = sb.tile([128, 512], BF16, name="warm_in", tag="warm_in")
        nc.vector.memset(warm_in[:], 1.0)
        warm_ps = ps.tile([128, 512], F32, name="warm_ps", tag="s")
        for i in range(6):
            nc.tensor.matmul(
                warm_ps[:],
                lhsT=warm_in[:, 0:128],
                rhs=warm_in[:],
                start=(i == 0),
                stop=(i == 5),
            )

        # v(0) rides as head 0's first filler — S(0)/S(1) only need
        # qkT[0]/qkT[6], so head 0 starts without waiting on v weights.
        # The k-side copies run on ScalarE so the m0/m6 staging copies
        # proceed two at a time.
        emit_qk_h(0, 0)
        emit_qk_h(6, 0, copy_on_scalar=True)
        emit_qk_h(0, 1)
        emit_qk_h(6, 1, copy_on_scalar=True)

        # Remaining qkv work rides inside the attention stream as PE
        # filler during exp waits: head 0 carries the other v chunks
        # (needed from its own P@V loop onward); later heads each carry
        # one q/k chunk, landing one pair ahead of first use.
        QK_FILL = {1: (1, 7), 2: (2,), 3: (8,), 4: (3,), 5: (9,),
                   6: (4,), 7: (10,), 8: (5,), 9: (11,)}

        pending = None
        for h in range(NH):
            fillers = []
            if h == 0:
                fillers.append(lambda: [emit_v(n) for n in range(NQ)])
            for m in QK_FILL.get(h, ()):
                fillers.append(lambda m=m: emit_qk(m))
            if pending is not None:
                ph, un, bc = pending
                fillers.append(lambda ph=ph, un=un, bc=bc: emit_norm_post(ph, un, bc))

            def filler():
                for f in fillers:
                    f()

            oa = emit_head(h, filler=filler)
            pending = (h, *emit_norm_pre(h, oa, direct=(h == NH - 1)))
        emit_norm_post(*pending)

        # ---- output projection ---------------------------------------
        acc.release()
        ps.release()
        yps = tc.alloc_tile_pool(name="yps", bufs=3, space="PSUM")

        # The first three n-chunks accumulate c=0..4 up front — ~5us of
        # matmuls independent of attn_bf[5] that absorb head 11's
        # normalize-chain latency — and get their c=5 closer deferred.
        # By n=3 the chain has resolved, so the rest run as plain
        # 6-chunk accumulations with the bias-add + store pipelined
        # per-chunk behind them: the kernel tail is one chunk deep.
        y_tiles = {}
        for n in range(3):
            y_ps = yps.tile([128, C], F32, name=f"y_ps{n}", tag="y_ps")
            y_tiles[n] = y_ps
            for lo, hi in ((0, 512), (512, 768)):
                for c in range(CK - 1):
                    nc.tensor.matmul(
                        y_ps[:, lo:hi],
                        lhsT=attn_bf[c][:, n * 128:(n + 1) * 128],
                        rhs=wp_all[:, c, lo:hi],
                        start=(c == 0),
                        stop=False,
                    )
        for n in range(NQ):
            if n < 3:
                y_ps = y_tiles[n]
                for lo, hi in ((0, 512), (512, 768)):
                    nc.tensor.matmul(
                        y_ps[:, lo:hi],
                        lhsT=attn_bf[CK - 1][:, n * 128:(n + 1) * 128],
                        rhs=wp_all[:, CK - 1, lo:hi],
                        start=False,
                        stop=True,
                    )
            else:
                y_ps = yps.tile([128, C], F32, name=f"y_ps{n}", tag="y_ps")
                for lo, hi in ((0, 512), (512, 768)):
                    for c in range(CK):
                        nc.tensor.matmul(
                            y_ps[:, lo:hi],
                            lhsT=attn_bf[c][:, n * 128:(n + 1) * 128],
                            rhs=wp_all[:, c, lo:hi],
                            start=(c == 0),
                            stop=(c == CK - 1),
                        )
            y_sb = yst.tile([128, C], F32, name=f"y_sb{n}", tag="y")
            nc.vector.tensor_add(y_sb[:], y_ps[:], bias_bc[:])
            nc.sync.dma_start(out=out[n * 128:(n + 1) * 128, :], in_=y_sb[:])
        yps.release()


def build_graph():
    nc = bacc.Bacc("TRN2", target_bir_lowering=False, debug=False)
    xT = nc.declare_dram_parameter("xT", [C, N], BF16, isOutput=False)
    wqkvT = nc.declare_dram_parameter("wqkvT", [128, CK * 3 * C], BF16, isOutput=False)
    wprojT = nc.declare_dram_parameter("wprojT", [128, CK * C], BF16, isOutput=False)
    bproj = nc.declare_dram_parameter("bproj", [128, C], F32, isOutput=False)
    out = nc.declare_dram_parameter("out", [N, C], F32, isOutput=True)
    with tile.TileContext(nc) as tc:
        _emit(tc, xT.ap(), wqkvT.ap(), wprojT.ap(), bproj.ap(), out.ap())
    nc.compile()
    return nc


_GRAPH = None


def _get_graph():
    global _GRAPH
    if _GRAPH is None:
        _GRAPH = build_graph()
    return _GRAPH


def make_in_maps(x, W_qkv, W_proj, b_proj):
    import ml_dtypes

    bf16 = ml_dtypes.bfloat16
    x = np.asarray(x, dtype=np.float32)
    wqkvT = np.asarray(W_qkv, dtype=np.float32).T.astype(bf16)  # [768, 2304]
    # permute columns into priority-contiguous groups:
    # [m0 | m6 | m1 m2 | m7 m8 | v | m3 m4 m5 | m9 m10 m11] (see wcol)
    perm = np.concatenate([
        np.arange(0, 128),          # q m0
        np.arange(768, 896),        # k m6
        np.arange(128, 384),        # q m1,m2
        np.arange(896, 1152),       # k m7,m8
        np.arange(1536, 2304),      # v
        np.arange(384, 768),        # q m3..5
        np.arange(1152, 1536),      # k m9..11
    ])
    wqkvT = wqkvT[:, perm]
    # merge to partition-major [128, c, cols] so each priority class is
    # one strided DMA on device
    wqkvT = np.ascontiguousarray(
        wqkvT.reshape(CK, 128, 3 * C).transpose(1, 0, 2).reshape(128, CK * 3 * C)
    )
    wprojT = np.asarray(W_proj, dtype=np.float32).T.astype(bf16)
    wprojT = np.ascontiguousarray(
        wprojT.reshape(CK, 128, C).transpose(1, 0, 2).reshape(128, CK * C)
    )
    bp = np.ascontiguousarray(
        np.broadcast_to(np.asarray(b_proj, dtype=np.float32), (128, C))
    )
    xT_all = np.ascontiguousarray(x.transpose(0, 2, 1).astype(bf16))
    return [
        {"xT": xT_all[i], "wqkvT": wqkvT, "wprojT": wprojT, "bproj": bp}
        for i in range(B)
    ]


def run(x, W_qkv, W_proj, b_proj, trace=False):
    nc = _get_graph()
    in_maps = make_in_maps(x, W_qkv, W_proj, b_proj)
    res = run_bass_kernel_spmd(nc, in_maps, core_ids=list(range(B)), trace=trace)
    out = np.stack([res.results[i]["out"] for i in range(B)], axis=0)
    return out.astype(np.float32, copy=False), res


def kernel(x, W_qkv, W_proj, b_proj, H=None, W=None):
    out, _ = run(x, W_qkv, W_proj, b_proj)
    return out


# revision 46
# speedup vs baseline: 1.1870x; 1.0005x over previous
"""Multi-head attention (B=8, N=1024, C=768, 12 heads) on 8 TRN2 NeuronCores.

Sharding: data-parallel over batch — batch element b runs on core b, weights
replicated, zero collectives.

Per-core kernel (all matmuls bf16 on the TensorEngine):
  - Host pre-transposes AND pre-converts x, W_qkv, W_proj to bf16, permutes
    W_qkv columns into DMA-priority order, merges the weight chunks into
    partition-major blocks, and replicates the bias to 128 partitions.
    Tiles then DMA straight into matmul-ready SBUF layout: no on-device
    casts, no staging bounce, half the input DMA bytes, and few DMA
    instructions (each dma_start costs ~600ns of DGE issue and one
    semaphore that the end-of-graph drain walks). Numerically identical to
    casting on device — every matmul consumed bf16 anyway.
  - DMAs are issued in consumption order ([x(c)+W m0|m6] -> v -> other
    left blocks -> right blocks -> W_proj -> bias), x from the Scalar
    engine's DGE and W from Sync so descriptor generation runs in
    parallel. A 6-matmul scratch burst ramps the PE clock through the
    DMA lead-in (cold matmuls run at half clock for several us).
  - qkv: q^T,k^T [768,1024] and v [1024,768] via 6-chunk K=768 matmuls.
  - scores are computed TRANSPOSED per head: S^T[k,q] with lhsT=k^T-block,
    rhs=q^T-block, so the exp output P^T feeds the P@V matmul directly as
    the moving operand (no transpose of the attention matrix). The
    1/sqrt(d) scale rides for free on the exp's affine pre-scale.
  - softmax denominators come free: v is stored with a ones-column
    appended per head (lhsT [128,65]); row 64 of the P@V accumulator is
    sum_k exp(S), i.e. the denominator.
  - normalization runs entirely off the TensorEngine's critical path:
    copy the accumulator to SBUF in halves (releasing its PSUM slot as
    early as possible — it gates the next head's P@V), fast approximate
    reciprocal on VectorE, broadcast across partitions on the (otherwise
    idle) GpSimd engine, one elementwise multiply per head. The last
    head multiplies straight out of PSUM with broadcast and multiply
    split in halves — that chain gates the projection closers.
  - qkv chunk emission is interleaved into the attention stream as
    filler so the TensorEngine stays dense while ScalarE works through
    the exps.
  - proj: y = attn @ W_proj^T + b_proj. The first three n-chunks
    accumulate c=0..4 up front (PE work independent of head 11,
    absorbing its normalize-chain latency) with deferred c=5 closers;
    the rest run as plain 6-chunk accumulations with the bias-add +
    store pipelined per-chunk, so the kernel tail is one chunk deep.
"""

from contextlib import ExitStack

import numpy as np

import concourse.mybir as mybir
import concourse.tile as tile
from concourse import bacc
from concourse.bass_utils import run_bass_kernel_spmd

B, N, C = 8, 1024, 768
NH, D = 12, 64
CK = C // 128  # 6 contraction chunks of 128
NQ = N // 128  # 8 position chunks of 128
SCALE = D ** -0.5
F32 = mybir.dt.float32
BF16 = mybir.dt.bfloat16
Exp = mybir.ActivationFunctionType.Exp


def _emit(tc, xT, wqkvT, wprojT, bproj, out):
    nc = tc.nc
    with ExitStack() as ctx:
        sb = ctx.enter_context(tc.tile_pool(name="sb", bufs=1))
        pp = ctx.enter_context(tc.tile_pool(name="pp", bufs=8))
        small = ctx.enter_context(tc.tile_pool(name="small", bufs=2))
        yst = ctx.enter_context(tc.tile_pool(name="yst", bufs=3))
        # PSUM pools are released by hand: qkv+attention use ps/acc, the
        # projection reuses the freed banks for a deeper y pipeline.
        ps = tc.alloc_tile_pool(name="ps", bufs=3, space="PSUM")
        acc = tc.alloc_tile_pool(name="acc", bufs=1, space="PSUM")

        # ---- input tiles (DMA straight into matmul layout) ------------
        # Weights are host-merged into partition-major [128, (c, cols)]
        # layouts so whole priority classes load with ONE strided DMA
        # (fewer DMAs -> fewer semaphores -> shorter end-of-graph drain
        # and less DGE issue traffic). Host also pre-permutes wqkv
        # columns into priority groups: [0:768) = q-left|k-left,
        # [768:1536) = v, [1536:2304) = q-right|k-right.
        xT_bf = [
            sb.tile([128, N], BF16, name=f"xT_bf{c}", tag=f"xT_bf{c}")
            for c in range(CK)
        ]
        wq_all = sb.tile([128, CK, 3 * C], BF16, name="wq_all", tag="wq_all")
        wp_all = sb.tile([128, CK, C], BF16, name="wp_all", tag="wp_all")
        wqkvT_v = wqkvT.rearrange("p (c k) -> p c k", k=3 * C)
        wprojT_v = wprojT.rearrange("p (c k) -> p c k", k=C)

        # Issue order == consumption order: q-left/k-left weights and x
        # feed the opening qkv chains (per-c DMAs so the first chain
        # starts after ~one chunk); v weights must land before the v
        # fillers in head 0; right halves ride through the first few
        # attention heads; W_proj and the bias are only needed at the
        # projection, ~150us in. x issues from the (otherwise idle)
        # Scalar DGE so x and W descriptor generation run in parallel.
        for c in range(CK):
            nc.scalar.dma_start(out=xT_bf[c][:], in_=xT[c * 128:(c + 1) * 128, :])
            nc.sync.dma_start(out=wq_all[:, c, 0:256], in_=wqkvT_v[:, c, 0:256])
        # v weights next (head 0's v fillers start ~22us in), in halves
        # so the first emit_v chain isn't gated on the full block
        nc.sync.dma_start(out=wq_all[:, :, 768:1152], in_=wqkvT_v[:, :, 768:1152])
        nc.sync.dma_start(out=wq_all[:, :, 1152:1536], in_=wqkvT_v[:, :, 1152:1536])
        for c in range(CK):
            nc.sync.dma_start(out=wq_all[:, c, 256:768], in_=wqkvT_v[:, c, 256:768])
        nc.sync.dma_start(out=wq_all[:, :, 1536:2304], in_=wqkvT_v[:, :, 1536:2304])
        nc.sync.dma_start(out=wp_all[:], in_=wprojT_v[:, :, :])
        # bias pre-replicated host-side to all 128 partitions
        bias_bc = sb.tile([128, C], F32, name="bias_bc", tag="bias_bc")
        nc.sync.dma_start(out=bias_bc[:], in_=bproj[:, :])

        # ---- qkv projections -----------------------------------------
        # q^T,k^T: chunk m covers rows [m*128,(m+1)*128) of qkv^T;
        # m in 0..5 -> q, m in 6..11 -> k.
        qkT = [
            sb.tile([128, N], BF16, name=f"qkT{m}", tag=f"qkT{m}")
            for m in range(12)
        ]

        # column offset of qkv^T row-block m in the host-permuted W
        # layout: [m0 | m6 | m1 m2 | m7 m8 | v | m3 m4 m5 | m9 m10 m11]
        # (m0+m6 lead so the first DMA wave carries exactly the two
        # blocks head 0 needs)
        def wcol(m):
            if m == 0:
                return 0
            if m == 6:
                return 128
            if m in (1, 2):
                return 256 + (m - 1) * 128
            if m in (7, 8):
                return 512 + (m - 7) * 128
            if m < 6:
                return 1536 + (m - 3) * 128  # q-right
            return 1920 + (m - 9) * 128  # k-right

        def emit_qk_h(m, qh, copy_on_scalar=False, warm_fill=0):
            qk_ps = ps.tile([128, 512], F32, name=f"qk_ps{m}_{qh}", tag="s")
            for c in range(CK):
                nc.tensor.matmul(
                    qk_ps[:],
                    lhsT=wq_all[:, c, wcol(m):wcol(m) + 128],
                    rhs=xT_bf[c][:, qh * 512:(qh + 1) * 512],
                    start=(c == 0),
                    stop=(c == CK - 1),
                )
                if c < warm_fill:
                    # scratch matmul between DMA-paced chain links: keeps
                    # the PE busy (clock ramping) while the next x chunk
                    # streams in
                    nc.tensor.matmul(
                        warm_ps[:],
                        lhsT=warm_in[:, 0:128],
                        rhs=warm_in[:],
                        start=True,
                        stop=True,
                    )
            if copy_on_scalar:
                # ScalarE is idle until the first exp; its Copy is also
                # faster than DVE's here, halving the PSUM->SBUF chain
                # that gates head 0's first scores.
                nc.scalar.copy(qkT[m][:, qh * 512:(qh + 1) * 512], qk_ps[:])
            else:
                nc.vector.tensor_copy(qkT[m][:, qh * 512:(qh + 1) * 512], qk_ps[:])

        def emit_qk(m):
            for qh in range(2):
                emit_qk_h(m, qh)

        # v in natural layout [n, (head, d)] with a ones column appended
        # per head: v_sb[n] is [128, NH, D+1], [:, h, D] == 1.0.
        v_sb = [
            sb.tile([128, NH, D + 1], BF16, name=f"v_sb{n}", tag=f"v_sb{n}")
            for n in range(NQ)
        ]

        def emit_v(n):
            nc.vector.memset(v_sb[n][:, :, D], 1.0)
            for half in range(2):
                v_ps = ps.tile([128, 384], F32, name=f"v_ps{n}_{half}", tag="s")
                for c in range(CK):
                    nc.tensor.matmul(
                        v_ps[:],
                        lhsT=xT_bf[c][:, n * 128:(n + 1) * 128],
                        rhs=wq_all[:, c, 768 + half * 384:768 + (half + 1) * 384],
                        start=(c == 0),
                        stop=(c == CK - 1),
                    )
                nc.vector.tensor_copy(
                    v_sb[n][:, half * 6:(half + 1) * 6, 0:D],
                    v_ps[:].rearrange("p (h d) -> p h d", d=D),
                )

        # ---- attention ------------------------------------------------
        attn_bf = [
            sb.tile([128, N], BF16, name=f"attn_bf{p}", tag=f"attn_bf{p}")
            for p in range(6)
        ]

        def emit_head(h, filler=None):
            """S^T + exp + P@V for head h; `filler` emits extra PE work
            early in the stream (previous head's deferred normalize, next
            qkv chunk) so PE has exp-independent work while ScalarE runs."""
            q_tile = qkT[h // 2]
            k_tile = qkT[6 + h // 2]
            ro = (h % 2) * 64
            out_aug = acc.tile([D + 1, N], F32, name=f"oaug{h}", tag="acc")

            def emit_S(kc):
                st = ps.tile([128, N], F32, name=f"s{h}_{kc}", tag="s")
                for qh in range(2):
                    nc.tensor.matmul(
                        st[:, qh * 512:(qh + 1) * 512],
                        lhsT=k_tile[ro:ro + D, kc * 128:(kc + 1) * 128],
                        rhs=q_tile[ro:ro + D, qh * 512:(qh + 1) * 512],
                        start=True,
                        stop=True,
                    )
                pt = pp.tile([128, N], BF16, name=f"P{h}_{kc}", tag="P")
                nc.scalar.activation(pt[:], st[:], Exp, scale=SCALE)
                return pt

            def emit_v_mm(kc, pt):
                for qh in range(2):
                    nc.tensor.matmul(
                        out_aug[:, qh * 512:(qh + 1) * 512],
                        lhsT=v_sb[kc][:, h, :],
                        rhs=pt[:, qh * 512:(qh + 1) * 512],
                        start=(kc == 0),
                        stop=(kc == NQ - 1),
                    )

            # software pipeline: exp(kc) overlaps S(kc+1) and P@V(kc-1)
            pts = {0: emit_S(0), 1: emit_S(1)}
            if filler is not None:
                filler()
            for kc in range(NQ):
                emit_v_mm(kc, pts.pop(kc))
                if kc + 2 < NQ:
                    pts[kc + 2] = emit_S(kc + 2)
            return out_aug

        def emit_norm_pre(h, oa, direct=False):
            """DVE/GpSimd-only part: reciprocal chain first (it gates the
            final multiply), then stage the accumulator to SBUF to release
            its PSUM slot. For the last head (`direct`) the multiply reads
            the accumulator straight from PSUM instead — shortest tail."""
            if not direct:
                # staging copy FIRST: it releases the single-slot PSUM
                # accumulator, which gates the next head's P@V matmuls.
                # Split in halves so the first starts right after the
                # P@V qh0 matmul, before qh1 has finished.
                un = small.tile([D, N], F32, name=f"un{h}", tag="un")
                nc.vector.tensor_copy(un[:, 0:512], oa[0:D, 0:512])
                nc.vector.tensor_copy(un[:, 512:N], oa[0:D, 512:N])
            dn = small.tile([1, N], F32, name=f"dn{h}", tag="dn")
            nc.vector.tensor_copy(dn[:], oa[D:D + 1, :])
            rc = small.tile([1, N], F32, name=f"rc{h}", tag="rc")
            # reciprocal_approx_fast's uOp program only works from
            # partition 0 on hardware, hence the dn bounce copy above.
            nc.vector.reciprocal_approx_fast(rc[:], dn[:])
            rcb = small.tile([1, N], BF16, name=f"rcb{h}", tag="rcb")
            nc.vector.tensor_copy(rcb[:], rc[:])
            bcast = small.tile([64, N], BF16, name=f"bcast{h}", tag="bcast")
            if direct:
                # last head: the chain below gates the proj closers, so
                # pipeline broadcast and multiply in halves
                nc.gpsimd.partition_broadcast(bcast[:, 0:512], rcb[:, 0:512])
                nc.gpsimd.partition_broadcast(bcast[:, 512:N], rcb[:, 512:N])
                return oa, bcast
            nc.gpsimd.partition_broadcast(bcast[:], rcb[:])
            return un, bcast

        def emit_norm_post(h, un, bcast):
            p, ro = h // 2, (h % 2) * 64
            if h == NH - 1:
                for lo, hi in ((0, 512), (512, N)):
                    nc.vector.tensor_mul(
                        attn_bf[p][ro:ro + 64, lo:hi], un[0:D, lo:hi], bcast[:, lo:hi]
                    )
            else:
                nc.vector.tensor_mul(attn_bf[p][ro:ro + 64, :], un[0:D, :], bcast[:])

        # ---- PE p-state warm-up --------------------------------------
        # The PE clock ramps with continuous busy time; the first ~5us
        # of the kernel are DMA lead-in with nothing else for the PE to
        # do, so a short scratch-matmul burst there ramps the clock
        # before the first real chains arrive (cold matmuls run ~2.7x
        # slow). Seeded by a DVE memset — no DMA dependency.
        warm_in = sb.tile([128, 512], BF16, name="warm_in", tag="warm_in")
        nc.vector.memset(warm_in[:], 1.0)
        warm_ps = ps.tile([128, 512], F32, name="warm_ps", tag="s")
        for i in range(6):
            nc.tensor.matmul(
                warm_ps[:],
                lhsT=warm_in[:, 0:128],
                rhs=warm_in[:],
                start=(i == 0),
                stop=(i == 5),
            )

        # v(0) rides as head 0's first filler — S(0)/S(1) only need
        # qkT[0]/qkT[6], so head 0 starts without waiting on v weights.
        # The k-side copies run on ScalarE so the m0/m6 staging copies
        # proceed two at a time.
        emit_qk_h(0, 0)
        emit_qk_h(6, 0, copy_on_scalar=True)
        emit_qk_h(0, 1)
        emit_qk_h(6, 1, copy_on_scalar=True)

        # Remaining qkv work rides inside the attention stream as PE
        # filler during exp waits: head 0 carries the other v chunks
        # (needed from its own P@V loop onward); later heads each carry
        # one q/k chunk, landing one pair ahead of first use.
        QK_FILL = {1: (1, 7), 2: (2,), 3: (8,), 4: (3,), 5: (9,),
                   6: (4,), 7: (10,), 8: (5,), 9: (11,)}

        pending = None
        for h in range(NH):
            fillers = []
            if h == 0:
                fillers.append(lambda: [emit_v(n) for n in range(NQ)])
            for m in QK_FILL.get(h, ()):
                fillers.append(lambda m=m: emit_qk(m))
            if pending is not None:
                ph, un, bc = pending
                fillers.append(lambda ph=ph, un=un, bc=bc: emit_norm_post(ph, un, bc))

            def filler():
                for f in fillers:
                    f()

            oa = emit_head(h, filler=filler)
            pending = (h, *emit_norm_pre(h, oa, direct=(h == NH - 1)))
        emit_norm_post(*pending)

        # ---- output projection ---------------------------------------
        acc.release()
        ps.release()
        yps = tc.alloc_tile_pool(name="yps", bufs=3, space="PSUM")
        # One extra accumulator on the banks acc just freed: the first
        # full 6-chunk chain (n=3) runs there, so it doesn't have to
        # wait for a deferred closer's bias-add to release a yps slot.
        yps2 = tc.alloc_tile_pool(name="yps2", bufs=1, space="PSUM")

        # The first three n-chunks accumulate c=0..4 up front — ~5us of
        # matmuls independent of attn_bf[5] that absorb head 11's
        # normalize-chain latency — and get their c=5 closer deferred.
        # By n=3 the chain has resolved, so the rest run as plain
        # 6-chunk accumulations with the bias-add + store pipelined
        # per-chunk behind them: the kernel tail is one chunk deep.
        y_tiles = {}
        for n in range(3):
            y_ps = yps.tile([128, C], F32, name=f"y_ps{n}", tag="y_ps")
            y_tiles[n] = y_ps
            for lo, hi in ((0, 512), (512, 768)):
                for c in range(CK - 1):
                    nc.tensor.matmul(
                        y_ps[:, lo:hi],
                        lhsT=attn_bf[c][:, n * 128:(n + 1) * 128],
                        rhs=wp_all[:, c, lo:hi],
                        start=(c == 0),
                        stop=False,
                    )
        for n in range(NQ):
            if n < 3:
                y_ps = y_tiles[n]
                for lo, hi in ((0, 512), (512, 768)):
                    nc.tensor.matmul(
                        y_ps[:, lo:hi],
                        lhsT=attn_bf[CK - 1][:, n * 128:(n + 1) * 128],
                        rhs=wp_all[:, CK - 1, lo:hi],
                        start=False,
                        stop=True,
                    )
            else:
                pool = yps2 if n == 3 else yps
                y_ps = pool.tile([128, C], F32, name=f"y_ps{n}", tag="y_ps")
                for lo, hi in ((0, 512), (512, 768)):
                    for c in range(CK):
                        nc.tensor.matmul(
                            y_ps[:, lo:hi],
                            lhsT=attn_bf[c][:, n * 128:(n + 1) * 128],
                            rhs=wp_all[:, c, lo:hi],
                            start=(c == 0),
                            stop=(c == CK - 1),
                        )
            y_sb = yst.tile([128, C], F32, name=f"y_sb{n}", tag="y")
            nc.vector.tensor_add(y_sb[:], y_ps[:], bias_bc[:])
            nc.sync.dma_start(out=out[n * 128:(n + 1) * 128, :], in_=y_sb[:])
        yps2.release()
        yps.release()


def build_graph():
    nc = bacc.Bacc("TRN2", target_bir_lowering=False, debug=False)
    xT = nc.declare_dram_parameter("xT", [C, N], BF16, isOutput=False)
    wqkvT = nc.declare_dram_parameter("wqkvT", [128, CK * 3 * C], BF16, isOutput=False)
    wprojT = nc.declare_dram_parameter("wprojT", [128, CK * C], BF16, isOutput=False)
    bproj = nc.declare_dram_parameter("bproj", [128, C], F32, isOutput=False)
    out = nc.declare_dram_parameter("out", [N, C], F32, isOutput=True)
    with tile.TileContext(nc) as tc:
        _emit(tc, xT.ap(), wqkvT.ap(), wprojT.ap(), bproj.ap(), out.ap())
    nc.compile()
    return nc


_GRAPH = None


def _get_graph():
    global _GRAPH
    if _GRAPH is None:
        _GRAPH = build_graph()
    return _GRAPH


def make_in_maps(x, W_qkv, W_proj, b_proj):
    import ml_dtypes

    bf16 = ml_dtypes.bfloat16
    x = np.asarray(x, dtype=np.float32)
    wqkvT = np.asarray(W_qkv, dtype=np.float32).T.astype(bf16)  # [768, 2304]
    # permute columns into priority-contiguous groups:
    # [m0 | m6 | m1 m2 | m7 m8 | v | m3 m4 m5 | m9 m10 m11] (see wcol)
    perm = np.concatenate([
        np.arange(0, 128),          # q m0
        np.arange(768, 896),        # k m6
        np.arange(128, 384),        # q m1,m2
        np.arange(896, 1152),       # k m7,m8
        np.arange(1536, 2304),      # v
        np.arange(384, 768),        # q m3..5
        np.arange(1152, 1536),      # k m9..11
    ])
    wqkvT = wqkvT[:, perm]
    # merge to partition-major [128, c, cols] so each priority class is
    # one strided DMA on device
    wqkvT = np.ascontiguousarray(
        wqkvT.reshape(CK, 128, 3 * C).transpose(1, 0, 2).reshape(128, CK * 3 * C)
    )
    wprojT = np.asarray(W_proj, dtype=np.float32).T.astype(bf16)
    wprojT = np.ascontiguousarray(
        wprojT.reshape(CK, 128, C).transpose(1, 0, 2).reshape(128, CK * C)
    )
    bp = np.ascontiguousarray(
        np.broadcast_to(np.asarray(b_proj, dtype=np.float32), (128, C))
    )
    xT_all = np.ascontiguousarray(x.transpose(0, 2, 1).astype(bf16))
    return [
        {"xT": xT_all[i], "wqkvT": wqkvT, "wprojT": wprojT, "bproj": bp}
        for i in range(B)
    ]


def run(x, W_qkv, W_proj, b_proj, trace=False):
    nc = _get_graph()
    in_maps = make_in_maps(x, W_qkv, W_proj, b_proj)
    res = run_bass_kernel_spmd(nc, in_maps, core_ids=list(range(B)), trace=trace)
    out = np.stack([res.results[i]["out"] for i in range(B)], axis=0)
    return out.astype(np.float32, copy=False), res


def kernel(x, W_qkv, W_proj, b_proj, H=None, W=None):
    out, _ = run(x, W_qkv, W_proj, b_proj)
    return out
